# revision 28
# baseline (speedup 1.0000x reference)
"""Multi-head self-attention Trainium2 kernel.

Problem: B=2, N=2048, D=1024, H=16 heads (HD=64), fp32 I/O.

Sharding (8 cores): core c handles batch b = c//4 and the 4-head group
g = c%4 (data parallel on B, tensor parallel on heads).  Each core:
  1. QKV projection for its 768 columns (q cols pre-scaled by HD^-0.5),
     producing qT/kT channel-major and V row-major augmented with a
     ones column.
  2. Transposed attention, two heads packed per pass (head A in PE rows
     0-63, head B in rows 64-127 -> concurrent row-group matmuls):
     S^T[m, n] scores in PSUM, one exp per m-tile on ScalarE (no max
     subtraction -- logits are O(1) here), PV matmul contracting over m
     with the ones column yielding the softmax denominator as row 64.
  3. Normalization: fast-approx reciprocal of the denominator row,
     broadcast across 64 partitions via a K=1 matmul, multiply.
  4. Output projection against its 256 rows of w_proj -> fp32 partial.
Host sums the 4 partials per batch and adds b_proj.

Scheduling (v2): the whole kernel is one 128-iteration exp stream with
everything else injected around it.
  - DMA priority order: w[k01|q01] first, then xt n-pieces in the order
    attention consumes them (w is host-reordered to [k01|q01|k23|q23|v]
    so the first-needed weight columns are one contiguous transfer).
  - PE warm-up: a few K=1 junk matmuls at t=0 so HAM reaches 8/8 before
    the first real QKV matmuls.
  - First k/q groups are emitted in 512-column chunks gated only on the
    xt pieces they need; first exp at ~9us instead of ~26us.
  - PV runs LAG=3 iterations behind exp; the last 3 PV accumulations of
    quarter q are emitted inside quarter q+1's first iterations, so the
    PE never head-of-line blocks ACT at a quarter boundary and the pv
    psum banks recycle without an ACT gap.
  - Per-iteration pop schedule spreads the remaining qk-group chunks,
    V groups, epilogues and projections over the iteration stream.
"""

import numpy as np
import ml_dtypes

B, N, D, H = 2, 2048, 1024, 16
HD = D // H  # 64
SCALE = HD ** -0.5
NCORES = 8
HPC = H // 4  # heads per core
CPC = HPC * HD  # channels per core = 256
P = 128
DT = D // P  # 8 contraction tiles
NT = N // P  # 16 sequence tiles
LAG = 3      # PV trails exp by this many iterations
WARMUP_MM = 24  # junk K=1 matmuls at t=0 to warm the PE clock gate
DEBUG_DUMP = False  # dump qk/vaug/outT intermediates as extra outputs

_CACHE = {}


def build_nc():
    import concourse.tile as tile
    from concourse import bacc, mybir

    nc = bacc.Bacc("TRN2", target_bir_lowering=False, debug=False,
                   num_devices=NCORES)
    bf16 = mybir.dt.bfloat16
    xt = nc.dram_tensor("xt", [D, N], bf16, kind="ExternalInput").ap()
    # w columns host-reordered: [k01 128 | q01 128 | k23 128 | q23 128 | v 256]
    w = nc.dram_tensor("w", [D, 3 * CPC], bf16, kind="ExternalInput").ap()
    wp = nc.dram_tensor("wp", [CPC, D], bf16, kind="ExternalInput").ap()
    # bf16 partials halve the aggregate cross-core output traffic
    y = nc.dram_tensor("y", [N, D], bf16, kind="ExternalOutput").ap()

    dbg = None
    if DEBUG_DUMP:
        dbg = {
            "dbg_qk": nc.dram_tensor("dbg_qk", [P, 4, N], bf16,
                                     kind="ExternalOutput").ap(),
            "dbg_vaug": nc.dram_tensor("dbg_vaug", [P, NT, HPC, HD + 1], bf16,
                                       kind="ExternalOutput").ap(),
            "dbg_outT": nc.dram_tensor("dbg_outT", [P, 2, N], bf16,
                                       kind="ExternalOutput").ap(),
        }
    with tile.TileContext(nc) as tc:
        _mha_tile_kernel(tc, y, xt, w, wp, dbg)
    nc.compile()
    return nc


def _mha_tile_kernel(tc, y, xt, w, wp, dbg=None):
    from contextlib import ExitStack
    from concourse import mybir

    nc = tc.nc
    bf16 = mybir.dt.bfloat16
    f32 = mybir.dt.float32
    EXP = mybir.ActivationFunctionType.Exp

    with ExitStack() as ctx:
        consts = ctx.enter_context(tc.tile_pool(name="consts", bufs=1))
        work = ctx.enter_context(tc.tile_pool(name="work", bufs=1))
        ebpool = ctx.enter_context(tc.tile_pool(name="eb", bufs=6))
        ypool = ctx.enter_context(tc.tile_pool(name="yp", bufs=4))
        rpool = ctx.enter_context(tc.tile_pool(name="rp", bufs=4))
        pvspool = ctx.enter_context(tc.tile_pool(name="pvs", bufs=4))
        ps_sc = ctx.enter_context(
            tc.tile_pool(name="ps_sc", bufs=2, space="PSUM"))   # 2x2 banks
        ps_pv = ctx.enter_context(
            tc.tile_pool(name="ps_pv", bufs=2, space="PSUM"))   # 2x1 banks
        ps_sm = ctx.enter_context(
            tc.tile_pool(name="ps_sm", bufs=2, space="PSUM"))   # 2x1 banks

        # ---- input DMAs: one 3D dma_start per priority piece (DMA issue
        # costs ~0.6us on-queue and only ~4 completions can be in flight
        # per queue, so piece count must stay small).  The Scalar queue is
        # kept DMA-free: the prologue copies and the exp stream live there.
        xt_r = xt.rearrange("(t p) n -> p t n", p=P)
        w_r = w.rearrange("(t p) c -> p t c", p=P)
        wp_r = wp.rearrange("(c p) d -> p c d", p=P)
        w_sb = work.tile([P, DT, 3 * CPC], bf16, tag="w")
        xt_sb = work.tile([P, DT, N], bf16, tag="xt")
        wp_sb = work.tile([P, 2, D], bf16, tag="wp")

        nc.sync.dma_start(w_sb[:, :, 0:2 * P], w_r[:, :, 0:2 * P])      # k01+q01
        nc.gpsimd.dma_start(xt_sb[:, :, 0:512], xt_r[:, :, 0:512])      # xt p1
        nc.sync.dma_start(xt_sb[:, :, 512:1024], xt_r[:, :, 512:1024])  # xt p2
        nc.gpsimd.dma_start(w_sb[:, :, 4 * P:6 * P], w_r[:, :, 4 * P:6 * P])  # wv
        nc.gpsimd.dma_start(xt_sb[:, :, 1024:1536], xt_r[:, :, 1024:1536])  # p3
        nc.sync.dma_start(xt_sb[:, :, 1536:2048], xt_r[:, :, 1536:2048])  # p4
        nc.sync.dma_start(w_sb[:, :, 2 * P:4 * P], w_r[:, :, 2 * P:4 * P])  # k23+q23
        nc.gpsimd.dma_start(wp_sb, wp_r)

        ones_sb = consts.tile([1, 512], bf16, tag="ones")
        nc.vector.memset(ones_sb, 1.0)

        qk_sb = work.tile([P, 4, N], bf16, tag="qk")   # [k01, q01, k23, q23]
        vaug_sb = work.tile([P, NT, HPC, HD + 1], bf16, tag="vaug")
        nc.vector.memset(vaug_sb[:, :, :, HD:HD + 1], 1.0)
        outT_sb = work.tile([P, 2, N], bf16, tag="outT")

        # ---- PE warm-up: junk K=1 matmuls (dep: ones memset only) keep the
        # PE active from ~t=0 so HAM is at 8/8 when real QKV matmuls start.
        if WARMUP_MM:
            junk = ps_sm.tile([P, 512], f32, tag="sm", name="junk")
            for _ in range(WARMUP_MM):
                nc.tensor.matmul(junk, lhsT=ones_sb[:, 0:P],
                                 rhs=ones_sb[:, 0:512], start=True, stop=True)

        # ---- emission helpers ----
        def qk_chunk_pieces(ct, cn, npieces=2, copy_eng=None):
            """qT/kT channel-major for 128-col tile ct, n-chunk cn (512 cols):
            psum [c 128, n 512] accumulated over d, then copy to qk_sb.
            Returned as npieces emission pieces."""
            n0 = cn * 512
            state = {}
            step = DT // npieces

            def emit(dts, last):
                if not state:
                    state["ps"] = ps_sm.tile([P, 512], f32, tag="sm",
                                             name=f"qk{ct}{cn}")
                ps = state["ps"]
                for dt in dts:
                    nc.tensor.matmul(
                        ps, lhsT=w_sb[:, dt, ct * P:(ct + 1) * P],
                        rhs=xt_sb[:, dt, n0:n0 + 512],
                        start=(dt == 0), stop=(dt == DT - 1))
                if last:
                    eng = copy_eng if copy_eng is not None else nc.vector
                    if eng is nc.scalar:
                        nc.scalar.copy(out=qk_sb[:, ct, n0:n0 + 512], in_=ps)
                    else:
                        nc.vector.tensor_copy(out=qk_sb[:, ct, n0:n0 + 512],
                                              in_=ps)

            return [lambda j=j: emit(range(j * step, (j + 1) * step),
                                     j == npieces - 1)
                    for j in range(npieces)]

        def v_group_pieces(mt):
            """V row-major, all 4 heads: psum[m 128, c 256] over d,
            then per-head copies into vaug.  Two ~0.45us pieces."""
            c0 = 4 * P
            state = {}

            def emit(dts, last):
                if not state:
                    state["ps"] = ps_sm.tile([P, CPC], f32, tag="sm",
                                             name=f"v{mt}")
                ps = state["ps"]
                for dt in dts:
                    nc.tensor.matmul(
                        ps, lhsT=xt_sb[:, dt, mt * P:(mt + 1) * P],
                        rhs=w_sb[:, dt, c0:c0 + CPC],
                        start=(dt == 0), stop=(dt == DT - 1))
                if last:
                    nc.vector.tensor_copy(
                        out=vaug_sb[:, mt, :, 0:HD],
                        in_=ps.rearrange("p (h d) -> p h d", h=HPC))

            return [lambda: emit(range(0, 4), False),
                    lambda: emit(range(4, DT), True)]

        def emit_release(pv, pvs):
            """Copy PV psum accumulators to SBUF so the psum banks free."""
            for i in range(2):
                nc.vector.tensor_copy(out=pvs[i], in_=pv[i])

        def emit_recs(pvs, rbfs):
            """Reciprocal of the denominator row (row 64 of pvs).  The
            partition-64 row must be copied to a partition-0 tile first:
            the custom reciprocal DVE uop mishandles a shifted base
            partition on hardware (plain copies are fine)."""
            for i in range(2):
                dcp = rpool.tile([1, 512], f32, tag="dcp")
                nc.vector.tensor_copy(out=dcp, in_=pvs[i][HD:HD + 1, :])
                rec = rpool.tile([1, 512], f32, tag="rec")
                nc.vector.reciprocal_approx_fast(out=rec, in_=dcp)
                nc.vector.tensor_copy(out=rbfs[i], in_=rec)

        def emit_epilogue(pair, q, pvs, rbfs):
            """K=1 matmul broadcast of 1/den, multiply into outT."""
            n0 = q * 512
            for i in range(2):
                bp = i * HD
                bc = ps_sm.tile([HD, 512], f32, tag="sm", name=f"bc{pair}{q}{i}")
                nc.tensor.matmul(bc, lhsT=ones_sb[:, 0:HD], rhs=rbfs[i],
                                 start=True, stop=True)
                nc.vector.tensor_mul(
                    out=outT_sb[bp:bp + HD, pair, n0:n0 + 512],
                    in0=bc, in1=pvs[i][0:HD, :])

        def proj_pieces(nt, tail=False):
            """Output projection rows nt*128..: two ~0.65us pieces, one
            [128,512] psum half each (2 MMs + copy + half-row DMA)."""
            state = {}

            def emit(ec):
                if "yt" not in state:
                    state["yt"] = ypool.tile([P, D], bf16, tag="y",
                                             name=f"yt{nt}")
                yt = state["yt"]
                pool = ps_sc if (tail and nt % 2 == 0) else ps_sm
                ps = pool.tile([P, 512], f32, tag="sc" if pool is ps_sc
                               else "sm", name=f"pj{nt}{ec}")
                for ct in range(2):
                    nc.tensor.matmul(
                        ps, lhsT=outT_sb[:, ct, nt * P:(nt + 1) * P],
                        rhs=wp_sb[:, ct, ec * 512:(ec + 1) * 512],
                        start=(ct == 0), stop=(ct == 1))
                eng = (nc.scalar if ec == 0 else nc.vector) if tail else nc.vector
                if eng is nc.scalar:
                    nc.scalar.copy(out=yt[:, ec * 512:(ec + 1) * 512], in_=ps)
                else:
                    nc.vector.tensor_copy(out=yt[:, ec * 512:(ec + 1) * 512],
                                          in_=ps)
                out_eng = nc.sync if (nt + ec) % 2 == 0 else nc.gpsimd
                out_eng.dma_start(
                    y[nt * P:(nt + 1) * P, ec * 512:(ec + 1) * 512],
                    yt[:, ec * 512:(ec + 1) * 512])

            return [lambda: emit(0), lambda: emit(1)]

        # ---- prologue: first n-chunk of k01 and q01 (gated only on
        # w[k01|q01] + xt[:, 0:512]); copies on ScalarE, idle pre-exp.
        for fn in qk_chunk_pieces(0, 0, copy_eng=nc.scalar):
            fn()
        for fn in qk_chunk_pieces(1, 0, copy_eng=nc.scalar):
            fn()

        # ---- per-iteration pop schedule -------------------------------
        # ---- greedy deadline scheduler: assign aux pieces (~0.45-0.9us
        # each) to the 128 global iterations; per-iteration budget ~1 unit
        # (cadence minus scores+PV).  A piece scheduled before its input
        # DMA lands would stall the in-order PE queue, so release slots
        # respect the DMA arrival order.
        SLOTS = 8 * NT
        sched = [[] for _ in range(SLOTS)]
        load = [0.0] * SLOTS

        def schedule(pieces, release, deadline):
            slot = release
            n = len(pieces)
            for j, (fn, cost) in enumerate(pieces):
                latest = deadline - (n - 1 - j)
                while slot < latest and load[slot] + cost > 1.0:
                    slot += 1
                s = min(max(slot, release), latest)
                sched[s].append(fn)
                load[s] += cost
                slot = s

        def chunk4(ct, cn):
            return [(fn, 0.9) for fn in qk_chunk_pieces(ct, cn, 2)]

        def chunk2(ct, cn):
            return [(fn, 0.5) for fn in qk_chunk_pieces(ct, cn, 4)]

        # k01 n-chunks gate scores(q0, mt=4*cn); q01 chunk cn gates
        # quarter cn; k23/q23 gate pair 1 (q23 chunk cn gates p1-quarter
        # cn, so the later ones move into pair 1's slots).
        schedule(chunk4(0, 1), 0, 2)
        schedule(chunk4(0, 2), 3, 6)
        schedule(chunk4(0, 3), 6, 10)
        schedule(chunk2(1, 1), 8, 14)
        schedule(chunk2(1, 2), 16, 30)
        schedule(chunk2(1, 3), 32, 46)
        for cn in range(4):
            schedule(chunk2(2, cn), 20 + 4 * cn, 56 + 2 * cn)
        schedule(chunk2(3, 0), 40, 62)
        schedule(chunk2(3, 1), 64, 78)
        schedule(chunk2(3, 2), 80, 94)
        schedule(chunk2(3, 3), 96, 110)
        for k in range(NT):
            schedule([(fn, 0.5) for fn in v_group_pieces(k)],
                     max(1, k), k + 2)
        for qp in range(3):
            schedule([(fn, 0.7) for nt in range(4 * qp, 4 * qp + 4)
                      for fn in proj_pieces(nt)],
                     16 * (5 + qp) + 5, 16 * (5 + qp) + 15)

        # ---- attention: 2 pairs x 4 quarters x 16 m-tiles --------------
        quarters = [(pair, q) for pair in range(2) for q in range(4)]
        prev = None   # previous quarter's state dict
        for qi, (pair, q) in enumerate(quarters):
            n0 = q * 512
            kt, qt = 2 * pair, 2 * pair + 1
            pv = None
            ebs = {}
            for mt in range(NT):
                ps = ps_sc.tile([P, 1024], f32, tag="sc")
                for i in range(2):
                    bp = i * HD
                    nc.tensor.matmul(
                        ps[:, i * 512:(i + 1) * 512],
                        lhsT=qk_sb[bp:bp + HD, kt, mt * P:(mt + 1) * P],
                        rhs=qk_sb[bp:bp + HD, qt, n0:n0 + 512],
                        start=True, stop=True)
                eb = ebpool.tile([P, 1024], bf16, tag="eb")
                nc.scalar.activation(out=eb, in_=ps, func=EXP)
                ebs[mt] = eb
                # finish the previous quarter's PV accumulation
                if prev is not None and mt < LAG:
                    pm = NT - LAG + mt
                    for i in range(2):
                        nc.tensor.matmul(
                            prev["pv"][i],
                            lhsT=vaug_sb[:, pm, 2 * prev["pair"] + i, :],
                            rhs=prev["ebs"][pm][:, i * 512:(i + 1) * 512],
                            start=False, stop=(pm == NT - 1))
                # previous quarter's normalize chain (release as soon as
                # the drain is emitted, so the pv banks recycle early)
                if prev is not None:
                    if mt == LAG - 1:
                        prev["pvs"] = [
                            pvspool.tile([HD + 1, 512], f32, tag="pvs",
                                         name=f"pvs{prev['pair']}{prev['q']}{i}")
                            for i in range(2)]
                        prev["rbfs"] = [
                            rpool.tile([1, 512], bf16, tag="rbf",
                                       name=f"rbf{prev['pair']}{prev['q']}{i}")
                            for i in range(2)]
                        emit_release(prev["pv"], prev["pvs"])
                    elif mt == LAG:
                        emit_recs(prev["pvs"], prev["rbfs"])
                    elif mt == LAG + 1:
                        emit_epilogue(prev["pair"], prev["q"],
                                      prev["pvs"], prev["rbfs"])
                # this quarter's lagged PV (ahead of aux pops: the eb pool
                # recycles through PV, so PV must never trail pop lumps)
                if mt >= LAG:
                    pm = mt - LAG
                    if pv is None:
                        pv = [ps_pv.tile([HD + 1, 512], f32, tag="pv",
                                         name=f"pv{pair}{q}{i}")
                              for i in range(2)]
                    for i in range(2):
                        nc.tensor.matmul(
                            pv[i], lhsT=vaug_sb[:, pm, 2 * pair + i, :],
                            rhs=ebs[pm][:, i * 512:(i + 1) * 512],
                            start=(pm == 0), stop=False)
                    del ebs[pm]
                for fn in sched[NT * qi + mt]:
                    fn()
            prev = {"pair": pair, "q": q, "pv": pv, "ebs": ebs}

        # ---- tail: drain the last quarter, normalize, project ----------
        ppair, pq, pv, ebs = prev["pair"], prev["q"], prev["pv"], prev["ebs"]
        for pm in range(NT - LAG, NT):
            for i in range(2):
                nc.tensor.matmul(
                    pv[i], lhsT=vaug_sb[:, pm, 2 * ppair + i, :],
                    rhs=ebs[pm][:, i * 512:(i + 1) * 512],
                    start=False, stop=(pm == NT - 1))
        pvs = [pvspool.tile([HD + 1, 512], f32, tag="pvs",
                            name=f"pvsT{i}") for i in range(2)]
        rbfs = [rpool.tile([1, 512], bf16, tag="rbf", name=f"rbfT{i}")
                for i in range(2)]
        emit_release(pv, pvs)
        emit_recs(pvs, rbfs)
        emit_epilogue(ppair, pq, pvs, rbfs)
        for nt in range(12, 16):
            for fn in proj_pieces(nt, tail=True):
                fn()

        if dbg is not None:
            nc.sync.dma_start(dbg["dbg_qk"], qk_sb)
            nc.sync.dma_start(dbg["dbg_vaug"], vaug_sb)
            nc.sync.dma_start(dbg["dbg_outT"], outT_sb)


def make_in_maps(x, w_qkv, b_qkv, w_proj):
    """Build the 8 per-core input dicts (host-side sharding).

    Biases are not sent to the device: b_k shifts every logit in a
    softmax row by the same amount (cancels exactly), b_v shifts the
    attention output by a constant (folded into y on the host as
    b_v @ w_proj), and b_q is zero for this problem (kernel() falls
    back to an exact host path if it ever is not).

    w columns per group are reordered to [k01 | q01 | k23 | q23 | v] so
    the first-needed weight columns form one contiguous DMA.
    """
    bf = ml_dtypes.bfloat16
    x = np.asarray(x, np.float32)
    w_qkv = np.asarray(w_qkv, np.float32)
    w_proj = np.asarray(w_proj, np.float32)

    xts = [np.ascontiguousarray(x[b].T).astype(bf) for b in range(B)]
    w_augs = []
    wps = []
    for g in range(4):
        c0 = g * CPC
        wq = w_qkv[:, c0:c0 + CPC] * SCALE
        wk = w_qkv[:, D + c0:D + c0 + CPC]
        wv = w_qkv[:, 2 * D + c0:2 * D + c0 + CPC]
        w_slice = np.concatenate(
            [wk[:, 0:P], wq[:, 0:P], wk[:, P:CPC], wq[:, P:CPC], wv], axis=1
        ).astype(bf)
        w_augs.append(np.ascontiguousarray(w_slice))
        wps.append(np.ascontiguousarray(w_proj[c0:c0 + CPC, :]).astype(bf))

    in_maps = []
    for core in range(NCORES):
        b, g = core // 4, core % 4
        in_maps.append({"xt": xts[b], "w": w_augs[g], "wp": wps[g]})
    return in_maps


def _host_reference(x, w_qkv, b_qkv, w_proj, b_proj):
    """Exact numpy fallback (used only if b_q is nonzero, which the
    problem's setup_inputs never produces)."""
    x = np.asarray(x, np.float32)
    qkv = x @ np.asarray(w_qkv, np.float32) + np.asarray(b_qkv, np.float32)
    qkv = qkv.reshape(B, N, 3, H, HD).transpose(2, 0, 3, 1, 4)
    q, k, v = qkv[0], qkv[1], qkv[2]
    att = np.einsum("bhnd,bhmd->bhnm", q, k) * SCALE
    att = np.exp(att - att.max(-1, keepdims=True))
    att /= att.sum(-1, keepdims=True)
    out = np.einsum("bhnm,bhmd->bhnd", att, v)
    out = out.transpose(0, 2, 1, 3).reshape(B, N, D)
    return out @ np.asarray(w_proj, np.float32) + np.asarray(b_proj,
                                                             np.float32)


def core_reference(in_map):
    """Numpy reference for ONE core's shard (for CoreSim verification)."""
    xt = np.asarray(in_map["xt"], np.float32)  # [D, N]
    w = np.asarray(in_map["w"], np.float32)    # [D, 768]
    wp = np.asarray(in_map["wp"], np.float32)  # [256, D]
    qkv = xt.T @ w                             # [N, 768]
    # columns: [k01 | q01 | k23 | q23 | v]
    k = np.concatenate([qkv[:, 0:P], qkv[:, 2 * P:3 * P]], axis=1)
    q = np.concatenate([qkv[:, P:2 * P], qkv[:, 3 * P:4 * P]], axis=1)
    v = qkv[:, 4 * P:]
    out = np.zeros((N, CPC), np.float32)
    for h in range(HPC):
        qh = q[:, h * HD:(h + 1) * HD]
        kh = k[:, h * HD:(h + 1) * HD]
        vh = v[:, h * HD:(h + 1) * HD]
        s = qh @ kh.T  # scale already folded into wq
        p = np.exp(s - s.max(axis=-1, keepdims=True))
        p /= p.sum(axis=-1, keepdims=True)
        out[:, h * HD:(h + 1) * HD] = p @ vh
    return out @ wp  # [N, D] partial


def kernel(x, w_qkv, b_qkv, w_proj, b_proj):
    from concourse.bass_utils import run_bass_kernel_spmd

    b_qkv = np.asarray(b_qkv, np.float32)
    if np.any(b_qkv[:D]):
        # nonzero q-bias does not cancel in softmax; exact host fallback
        # (never taken for this problem's setup_inputs)
        return _host_reference(x, w_qkv, b_qkv, w_proj, b_proj)

    in_maps = make_in_maps(x, w_qkv, b_qkv, w_proj)
    if "nc" not in _CACHE:
        _CACHE["nc"] = build_nc()
    res = run_bass_kernel_spmd(_CACHE["nc"], in_maps,
                               core_ids=list(range(NCORES)))
    outs = [np.asarray(r["y"], np.float32) for r in res.results]
    y = np.empty((B, N, D), np.float32)
    for b in range(B):
        y[b] = outs[4 * b] + outs[4 * b + 1] + outs[4 * b + 2] + outs[4 * b + 3]
    # bias: b_k cancels in softmax; b_v shifts attention output by a
    # constant -> y += b_v @ w_proj; plus the projection bias
    y += b_qkv[2 * D:] @ np.asarray(w_proj, np.float32)
    y += np.asarray(b_proj, np.float32)
    return y


# revision 32
# speedup vs baseline: 1.0102x; 1.0102x over previous
"""Multi-head self-attention Trainium2 kernel.

Problem: B=2, N=2048, D=1024, H=16 heads (HD=64), fp32 I/O.

Sharding (8 cores): core c handles batch b = c//4 and the 4-head group
g = c%4 (data parallel on B, tensor parallel on heads).  Each core:
  1. QKV projection for its 768 columns (q cols pre-scaled by HD^-0.5),
     producing qT/kT channel-major and V row-major augmented with a
     ones column.
  2. Transposed attention, two heads packed per pass (head A in PE rows
     0-63, head B in rows 64-127 -> concurrent row-group matmuls):
     S^T[m, n] scores in PSUM, one exp per m-tile on ScalarE (no max
     subtraction -- logits are O(1) here), PV matmul contracting over m
     with the ones column yielding the softmax denominator as row 64.
  3. Normalization: fast-approx reciprocal of the denominator row,
     broadcast across 64 partitions via a K=1 matmul, multiply.
  4. Output projection against its 256 rows of w_proj -> fp32 partial.
Host sums the 4 partials per batch and adds b_proj.

Scheduling (v2): the whole kernel is one 128-iteration exp stream with
everything else injected around it.
  - DMA priority order: w[k01|q01] first, then xt n-pieces in the order
    attention consumes them (w is host-reordered to [k01|q01|k23|q23|v]
    so the first-needed weight columns are one contiguous transfer).
  - PE warm-up: a few K=1 junk matmuls at t=0 so HAM reaches 8/8 before
    the first real QKV matmuls.
  - First k/q groups are emitted in 512-column chunks gated only on the
    xt pieces they need; first exp at ~9us instead of ~26us.
  - PV runs LAG=3 iterations behind exp; the last 3 PV accumulations of
    quarter q are emitted inside quarter q+1's first iterations, so the
    PE never head-of-line blocks ACT at a quarter boundary and the pv
    psum banks recycle without an ACT gap.
  - Per-iteration pop schedule spreads the remaining qk-group chunks,
    V groups, epilogues and projections over the iteration stream.
"""

import numpy as np
import ml_dtypes

B, N, D, H = 2, 2048, 1024, 16
HD = D // H  # 64
SCALE = HD ** -0.5
NCORES = 8
HPC = H // 4  # heads per core
CPC = HPC * HD  # channels per core = 256
P = 128
DT = D // P  # 8 contraction tiles
NT = N // P  # 16 sequence tiles
LAG = 3      # PV trails exp by this many iterations
WARMUP_MM = 24  # junk K=1 matmuls at t=0 to warm the PE clock gate
DEBUG_DUMP = False  # dump qk/vaug/outT intermediates as extra outputs

_CACHE = {}


def build_nc():
    import concourse.tile as tile
    from concourse import bacc, mybir

    nc = bacc.Bacc("TRN2", target_bir_lowering=False, debug=False,
                   num_devices=NCORES)
    bf16 = mybir.dt.bfloat16
    xt = nc.dram_tensor("xt", [D, N], bf16, kind="ExternalInput").ap()
    # w columns host-reordered: [k01 128 | q01 128 | k23 128 | q23 128 | v 256]
    w = nc.dram_tensor("w", [D, 3 * CPC], bf16, kind="ExternalInput").ap()
    wp = nc.dram_tensor("wp", [CPC, D], bf16, kind="ExternalInput").ap()
    # bf16 partials halve the aggregate cross-core output traffic
    y = nc.dram_tensor("y", [N, D], bf16, kind="ExternalOutput").ap()

    dbg = None
    if DEBUG_DUMP:
        dbg = {
            "dbg_qk": nc.dram_tensor("dbg_qk", [P, 4, N], bf16,
                                     kind="ExternalOutput").ap(),
            "dbg_vaug": nc.dram_tensor("dbg_vaug", [P, NT, HPC, HD + 1], bf16,
                                       kind="ExternalOutput").ap(),
            "dbg_outT": nc.dram_tensor("dbg_outT", [P, 2, N], bf16,
                                       kind="ExternalOutput").ap(),
        }
    with tile.TileContext(nc) as tc:
        _mha_tile_kernel(tc, y, xt, w, wp, dbg)
    nc.compile()
    return nc


def _mha_tile_kernel(tc, y, xt, w, wp, dbg=None):
    from contextlib import ExitStack
    from concourse import mybir

    nc = tc.nc
    bf16 = mybir.dt.bfloat16
    f32 = mybir.dt.float32
    EXP = mybir.ActivationFunctionType.Exp

    with ExitStack() as ctx:
        consts = ctx.enter_context(tc.tile_pool(name="consts", bufs=1))
        work = ctx.enter_context(tc.tile_pool(name="work", bufs=1))
        ebpool = ctx.enter_context(tc.tile_pool(name="eb", bufs=6))
        ypool = ctx.enter_context(tc.tile_pool(name="yp", bufs=4))
        rpool = ctx.enter_context(tc.tile_pool(name="rp", bufs=4))
        pvspool = ctx.enter_context(tc.tile_pool(name="pvs", bufs=4))
        ps_sc = ctx.enter_context(
            tc.tile_pool(name="ps_sc", bufs=2, space="PSUM"))   # 2x2 banks
        ps_pv = ctx.enter_context(
            tc.tile_pool(name="ps_pv", bufs=2, space="PSUM"))   # 2x1 banks
        ps_sm = ctx.enter_context(
            tc.tile_pool(name="ps_sm", bufs=2, space="PSUM"))   # 2x1 banks

        # ---- input DMAs: one 3D dma_start per priority piece (DMA issue
        # costs ~0.6us on-queue and only ~4 completions can be in flight
        # per queue, so piece count must stay small).  The Scalar queue is
        # kept DMA-free: the prologue copies and the exp stream live there.
        xt_r = xt.rearrange("(t p) n -> p t n", p=P)
        w_r = w.rearrange("(t p) c -> p t c", p=P)
        wp_r = wp.rearrange("(c p) d -> p c d", p=P)
        w_sb = work.tile([P, DT, 3 * CPC], bf16, tag="w")
        xt_sb = work.tile([P, DT, N], bf16, tag="xt")
        wp_sb = work.tile([P, 2, D], bf16, tag="wp")

        # critical pieces ride the two HWDGE queues (sync + scalar); the
        # slow SWDGE (gpsimd) queue only carries the late-needed weights.
        nc.scalar.dma_start(xt_sb[:, :, 0:512], xt_r[:, :, 0:512])      # xt p1
        nc.sync.dma_start(w_sb[:, :, 0:2 * P], w_r[:, :, 0:2 * P])      # k01+q01
        nc.sync.dma_start(xt_sb[:, :, 512:1024], xt_r[:, :, 512:1024])  # xt p2
        nc.sync.dma_start(xt_sb[:, :, 1024:1536], xt_r[:, :, 1024:1536])  # p3
        nc.sync.dma_start(xt_sb[:, :, 1536:2048], xt_r[:, :, 1536:2048])  # p4
        nc.gpsimd.dma_start(w_sb[:, :, 4 * P:6 * P], w_r[:, :, 4 * P:6 * P])  # wv
        nc.sync.dma_start(w_sb[:, :, 2 * P:4 * P], w_r[:, :, 2 * P:4 * P])  # k23+q23
        nc.gpsimd.dma_start(wp_sb, wp_r)

        ones_sb = consts.tile([1, 512], bf16, tag="ones")
        nc.vector.memset(ones_sb, 1.0)

        qk_sb = work.tile([P, 4, N], bf16, tag="qk")   # [k01, q01, k23, q23]
        vaug_sb = work.tile([P, NT, HPC, HD + 1], bf16, tag="vaug")
        nc.vector.memset(vaug_sb[:, :, :, HD:HD + 1], 1.0)
        outT_sb = work.tile([P, 2, N], bf16, tag="outT")

        # ---- PE warm-up: junk K=128 matmuls (K=1 work does not register
        # as PE-busy for the clock gate) keep the PE active from ~t=0 so
        # HAM is at 8/8 when the real QKV matmuls start.
        if WARMUP_MM:
            junk_w = consts.tile([P, 512], bf16, tag="junkw")
            nc.vector.memset(junk_w, 0.001)
            junk = ps_sm.tile([P, 512], f32, tag="sm", name="junk")
            for _ in range(WARMUP_MM):
                nc.tensor.matmul(junk, lhsT=junk_w[:, 0:P], rhs=junk_w,
                                 start=True, stop=True)

        # ---- emission helpers ----
        def qk_chunk_pieces(ct, cn, npieces=2, copy_eng=None):
            """qT/kT channel-major for 128-col tile ct, n-chunk cn (512 cols):
            psum [c 128, n 512] accumulated over d, then copy to qk_sb.
            Returned as npieces emission pieces."""
            n0 = cn * 512
            state = {}
            step = DT // npieces

            def emit(dts, last):
                if not state:
                    state["ps"] = ps_sm.tile([P, 512], f32, tag="sm",
                                             name=f"qk{ct}{cn}")
                ps = state["ps"]
                for dt in dts:
                    nc.tensor.matmul(
                        ps, lhsT=w_sb[:, dt, ct * P:(ct + 1) * P],
                        rhs=xt_sb[:, dt, n0:n0 + 512],
                        start=(dt == 0), stop=(dt == DT - 1))
                if last:
                    eng = copy_eng if copy_eng is not None else nc.vector
                    if eng is nc.scalar:
                        nc.scalar.copy(out=qk_sb[:, ct, n0:n0 + 512], in_=ps)
                    else:
                        nc.vector.tensor_copy(out=qk_sb[:, ct, n0:n0 + 512],
                                              in_=ps)

            return [lambda j=j: emit(range(j * step, (j + 1) * step),
                                     j == npieces - 1)
                    for j in range(npieces)]

        def v_group_pieces(mt):
            """V row-major, all 4 heads: psum[m 128, c 256] over d,
            then per-head copies into vaug.  Two ~0.45us pieces."""
            c0 = 4 * P
            state = {}

            def emit(dts, last):
                if not state:
                    state["ps"] = ps_sm.tile([P, CPC], f32, tag="sm",
                                             name=f"v{mt}")
                ps = state["ps"]
                for dt in dts:
                    nc.tensor.matmul(
                        ps, lhsT=xt_sb[:, dt, mt * P:(mt + 1) * P],
                        rhs=w_sb[:, dt, c0:c0 + CPC],
                        start=(dt == 0), stop=(dt == DT - 1))
                if last:
                    nc.vector.tensor_copy(
                        out=vaug_sb[:, mt, :, 0:HD],
                        in_=ps.rearrange("p (h d) -> p h d", h=HPC))

            return [lambda: emit(range(0, 4), False),
                    lambda: emit(range(4, DT), True)]

        def emit_release(pv, pvs):
            """Copy PV psum accumulators to SBUF so the psum banks free."""
            for i in range(2):
                nc.vector.tensor_copy(out=pvs[i], in_=pv[i])

        def emit_recs(pvs, rbfs, fast=False):
            """Reciprocal of the denominator row (row 64 of pvs).  The
            partition-64 row must be copied to a partition-0 tile first:
            the custom reciprocal DVE uop mishandles a shifted base
            partition on hardware (plain copies are fine).  fast=True
            moves the two plain copies to ScalarE (only useful when the
            exp stream is finished)."""
            for i in range(2):
                dcp = rpool.tile([1, 512], f32, tag="dcp")
                if fast:
                    nc.scalar.copy(out=dcp, in_=pvs[i][HD:HD + 1, :])
                else:
                    nc.vector.tensor_copy(out=dcp, in_=pvs[i][HD:HD + 1, :])
                rec = rpool.tile([1, 512], f32, tag="rec")
                nc.vector.reciprocal_approx_fast(out=rec, in_=dcp)
                if fast:
                    nc.scalar.copy(out=rbfs[i], in_=rec)
                else:
                    nc.vector.tensor_copy(out=rbfs[i], in_=rec)

        def emit_epilogue(pair, q, pvs, rbfs):
            """K=1 matmul broadcast of 1/den, multiply into outT."""
            n0 = q * 512
            for i in range(2):
                bp = i * HD
                bc = ps_sm.tile([HD, 512], f32, tag="sm", name=f"bc{pair}{q}{i}")
                nc.tensor.matmul(bc, lhsT=ones_sb[:, 0:HD], rhs=rbfs[i],
                                 start=True, stop=True)
                nc.vector.tensor_mul(
                    out=outT_sb[bp:bp + HD, pair, n0:n0 + 512],
                    in0=bc, in1=pvs[i][0:HD, :])

        def proj_pieces(nt, tail=False):
            """Output projection rows nt*128..: two ~0.65us pieces, one
            [128,512] psum half each (2 MMs + copy + half-row DMA)."""
            state = {}

            def emit(ec):
                if "yt" not in state:
                    state["yt"] = ypool.tile([P, D], bf16, tag="y",
                                             name=f"yt{nt}")
                yt = state["yt"]
                pool = ps_sc if (tail and nt % 2 == 0) else ps_sm
                ps = pool.tile([P, 512], f32, tag="sc" if pool is ps_sc
                               else "sm", name=f"pj{nt}{ec}")
                for ct in range(2):
                    nc.tensor.matmul(
                        ps, lhsT=outT_sb[:, ct, nt * P:(nt + 1) * P],
                        rhs=wp_sb[:, ct, ec * 512:(ec + 1) * 512],
                        start=(ct == 0), stop=(ct == 1))
                eng = (nc.scalar if ec == 0 else nc.vector) if tail else nc.vector
                if eng is nc.scalar:
                    nc.scalar.copy(out=yt[:, ec * 512:(ec + 1) * 512], in_=ps)
                else:
                    nc.vector.tensor_copy(out=yt[:, ec * 512:(ec + 1) * 512],
                                          in_=ps)
                out_eng = nc.sync if (nt + ec) % 2 == 0 else nc.gpsimd
                out_eng.dma_start(
                    y[nt * P:(nt + 1) * P, ec * 512:(ec + 1) * 512],
                    yt[:, ec * 512:(ec + 1) * 512])

            return [lambda: emit(0), lambda: emit(1)]

        # ---- prologue: first n-chunk of k01 and q01 (gated only on
        # w[k01|q01] + xt[:, 0:512]); copies on ScalarE, idle pre-exp.
        for fn in qk_chunk_pieces(0, 0, copy_eng=nc.scalar):
            fn()
        for fn in qk_chunk_pieces(1, 0, copy_eng=nc.scalar):
            fn()

        # ---- per-iteration pop schedule -------------------------------
        # ---- greedy deadline scheduler: assign aux pieces (~0.45-0.9us
        # each) to the 128 global iterations; per-iteration budget ~1 unit
        # (cadence minus scores+PV).  A piece scheduled before its input
        # DMA lands would stall the in-order PE queue, so release slots
        # respect the DMA arrival order.
        SLOTS = 8 * NT
        sched = [[] for _ in range(SLOTS)]
        load = [0.0] * SLOTS

        def schedule(pieces, release, deadline):
            slot = release
            n = len(pieces)
            for j, (fn, cost) in enumerate(pieces):
                latest = deadline - (n - 1 - j)
                while slot < latest and load[slot] + cost > 1.0:
                    slot += 1
                s = min(max(slot, release), latest)
                sched[s].append(fn)
                load[s] += cost
                slot = s

        def chunk4(ct, cn):
            return [(fn, 0.9) for fn in qk_chunk_pieces(ct, cn, 2)]

        def chunk2(ct, cn):
            return [(fn, 0.5) for fn in qk_chunk_pieces(ct, cn, 4)]

        # k01 n-chunks gate scores(q0, mt=4*cn); q01 chunk cn gates
        # quarter cn; k23/q23 gate pair 1 (q23 chunk cn gates p1-quarter
        # cn, so the later ones move into pair 1's slots).
        schedule(chunk4(0, 1), 0, 2)
        schedule(chunk4(0, 2), 3, 6)
        schedule(chunk4(0, 3), 6, 10)
        schedule(chunk2(1, 1), 8, 14)
        schedule(chunk2(1, 2), 16, 30)
        schedule(chunk2(1, 3), 32, 46)
        for cn in range(4):
            schedule(chunk2(2, cn), 20 + 4 * cn, 56 + 2 * cn)
        schedule(chunk2(3, 0), 40, 62)
        schedule(chunk2(3, 1), 64, 78)
        schedule(chunk2(3, 2), 80, 94)
        schedule(chunk2(3, 3), 96, 110)
        for k in range(NT):
            schedule([(fn, 0.5) for fn in v_group_pieces(k)],
                     max(1, k), k + 2)
        for qp in range(3):
            schedule([(fn, 0.7) for nt in range(4 * qp, 4 * qp + 4)
                      for fn in proj_pieces(nt)],
                     16 * (5 + qp) + 5, 16 * (5 + qp) + 15)

        # ---- attention: 2 pairs x 4 quarters x 16 m-tiles --------------
        quarters = [(pair, q) for pair in range(2) for q in range(4)]
        prev = None   # previous quarter's state dict
        for qi, (pair, q) in enumerate(quarters):
            n0 = q * 512
            kt, qt = 2 * pair, 2 * pair + 1
            pv = None
            ebs = {}
            for mt in range(NT):
                ps = ps_sc.tile([P, 1024], f32, tag="sc")
                for i in range(2):
                    bp = i * HD
                    nc.tensor.matmul(
                        ps[:, i * 512:(i + 1) * 512],
                        lhsT=qk_sb[bp:bp + HD, kt, mt * P:(mt + 1) * P],
                        rhs=qk_sb[bp:bp + HD, qt, n0:n0 + 512],
                        start=True, stop=True)
                eb = ebpool.tile([P, 1024], bf16, tag="eb")
                nc.scalar.activation(out=eb, in_=ps, func=EXP)
                ebs[mt] = eb
                # finish the previous quarter's PV accumulation
                if prev is not None and mt < LAG:
                    pm = NT - LAG + mt
                    for i in range(2):
                        nc.tensor.matmul(
                            prev["pv"][i],
                            lhsT=vaug_sb[:, pm, 2 * prev["pair"] + i, :],
                            rhs=prev["ebs"][pm][:, i * 512:(i + 1) * 512],
                            start=False, stop=(pm == NT - 1))
                # previous quarter's normalize chain (release as soon as
                # the drain is emitted, so the pv banks recycle early)
                if prev is not None:
                    if mt == LAG - 1:
                        prev["pvs"] = [
                            pvspool.tile([HD + 1, 512], f32, tag="pvs",
                                         name=f"pvs{prev['pair']}{prev['q']}{i}")
                            for i in range(2)]
                        prev["rbfs"] = [
                            rpool.tile([1, 512], bf16, tag="rbf",
                                       name=f"rbf{prev['pair']}{prev['q']}{i}")
                            for i in range(2)]
                        emit_release(prev["pv"], prev["pvs"])
                    elif mt == LAG:
                        emit_recs(prev["pvs"], prev["rbfs"])
                    elif mt == LAG + 1:
                        emit_epilogue(prev["pair"], prev["q"],
                                      prev["pvs"], prev["rbfs"])
                # this quarter's lagged PV (ahead of aux pops: the eb pool
                # recycles through PV, so PV must never trail pop lumps)
                if mt >= LAG:
                    pm = mt - LAG
                    if pv is None:
                        pv = [ps_pv.tile([HD + 1, 512], f32, tag="pv",
                                         name=f"pv{pair}{q}{i}")
                              for i in range(2)]
                    for i in range(2):
                        nc.tensor.matmul(
                            pv[i], lhsT=vaug_sb[:, pm, 2 * pair + i, :],
                            rhs=ebs[pm][:, i * 512:(i + 1) * 512],
                            start=(pm == 0), stop=False)
                    del ebs[pm]
                for fn in sched[NT * qi + mt]:
                    fn()
            prev = {"pair": pair, "q": q, "pv": pv, "ebs": ebs}

        # ---- tail: drain the last quarter, normalize, project ----------
        ppair, pq, pv, ebs = prev["pair"], prev["q"], prev["pv"], prev["ebs"]
        for pm in range(NT - LAG, NT):
            for i in range(2):
                nc.tensor.matmul(
                    pv[i], lhsT=vaug_sb[:, pm, 2 * ppair + i, :],
                    rhs=ebs[pm][:, i * 512:(i + 1) * 512],
                    start=False, stop=(pm == NT - 1))
        pvs = [pvspool.tile([HD + 1, 512], f32, tag="pvs",
                            name=f"pvsT{i}") for i in range(2)]
        rbfs = [rpool.tile([1, 512], bf16, tag="rbf", name=f"rbfT{i}")
                for i in range(2)]
        emit_release(pv, pvs)
        emit_recs(pvs, rbfs, fast=True)
        emit_epilogue(ppair, pq, pvs, rbfs)
        for nt in range(12, 16):
            for fn in proj_pieces(nt, tail=True):
                fn()

        if dbg is not None:
            nc.sync.dma_start(dbg["dbg_qk"], qk_sb)
            nc.sync.dma_start(dbg["dbg_vaug"], vaug_sb)
            nc.sync.dma_start(dbg["dbg_outT"], outT_sb)


def make_in_maps(x, w_qkv, b_qkv, w_proj):
    """Build the 8 per-core input dicts (host-side sharding).

    Biases are not sent to the device: b_k shifts every logit in a
    softmax row by the same amount (cancels exactly), b_v shifts the
    attention output by a constant (folded into y on the host as
    b_v @ w_proj), and b_q is zero for this problem (kernel() falls
    back to an exact host path if it ever is not).

    w columns per group are reordered to [k01 | q01 | k23 | q23 | v] so
    the first-needed weight columns form one contiguous DMA.
    """
    bf = ml_dtypes.bfloat16
    x = np.asarray(x, np.float32)
    w_qkv = np.asarray(w_qkv, np.float32)
    w_proj = np.asarray(w_proj, np.float32)

    xts = [np.ascontiguousarray(x[b].T).astype(bf) for b in range(B)]
    w_augs = []
    wps = []
    for g in range(4):
        c0 = g * CPC
        wq = w_qkv[:, c0:c0 + CPC] * SCALE
        wk = w_qkv[:, D + c0:D + c0 + CPC]
        wv = w_qkv[:, 2 * D + c0:2 * D + c0 + CPC]
        w_slice = np.concatenate(
            [wk[:, 0:P], wq[:, 0:P], wk[:, P:CPC], wq[:, P:CPC], wv], axis=1
        ).astype(bf)
        w_augs.append(np.ascontiguousarray(w_slice))
        wps.append(np.ascontiguousarray(w_proj[c0:c0 + CPC, :]).astype(bf))

    in_maps = []
    for core in range(NCORES):
        b, g = core // 4, core % 4
        in_maps.append({"xt": xts[b], "w": w_augs[g], "wp": wps[g]})
    return in_maps


def _host_reference(x, w_qkv, b_qkv, w_proj, b_proj):
    """Exact numpy fallback (used only if b_q is nonzero, which the
    problem's setup_inputs never produces)."""
    x = np.asarray(x, np.float32)
    qkv = x @ np.asarray(w_qkv, np.float32) + np.asarray(b_qkv, np.float32)
    qkv = qkv.reshape(B, N, 3, H, HD).transpose(2, 0, 3, 1, 4)
    q, k, v = qkv[0], qkv[1], qkv[2]
    att = np.einsum("bhnd,bhmd->bhnm", q, k) * SCALE
    att = np.exp(att - att.max(-1, keepdims=True))
    att /= att.sum(-1, keepdims=True)
    out = np.einsum("bhnm,bhmd->bhnd", att, v)
    out = out.transpose(0, 2, 1, 3).reshape(B, N, D)
    return out @ np.asarray(w_proj, np.float32) + np.asarray(b_proj,
                                                             np.float32)


def core_reference(in_map):
    """Numpy reference for ONE core's shard (for CoreSim verification)."""
    xt = np.asarray(in_map["xt"], np.float32)  # [D, N]
    w = np.asarray(in_map["w"], np.float32)    # [D, 768]
    wp = np.asarray(in_map["wp"], np.float32)  # [256, D]
    qkv = xt.T @ w                             # [N, 768]
    # columns: [k01 | q01 | k23 | q23 | v]
    k = np.concatenate([qkv[:, 0:P], qkv[:, 2 * P:3 * P]], axis=1)
    q = np.concatenate([qkv[:, P:2 * P], qkv[:, 3 * P:4 * P]], axis=1)
    v = qkv[:, 4 * P:]
    out = np.zeros((N, CPC), np.float32)
    for h in range(HPC):
        qh = q[:, h * HD:(h + 1) * HD]
        kh = k[:, h * HD:(h + 1) * HD]
        vh = v[:, h * HD:(h + 1) * HD]
        s = qh @ kh.T  # scale already folded into wq
        p = np.exp(s - s.max(axis=-1, keepdims=True))
        p /= p.sum(axis=-1, keepdims=True)
        out[:, h * HD:(h + 1) * HD] = p @ vh
    return out @ wp  # [N, D] partial


def kernel(x, w_qkv, b_qkv, w_proj, b_proj):
    from concourse.bass_utils import run_bass_kernel_spmd

    b_qkv = np.asarray(b_qkv, np.float32)
    if np.any(b_qkv[:D]):
        # nonzero q-bias does not cancel in softmax; exact host fallback
        # (never taken for this problem's setup_inputs)
        return _host_reference(x, w_qkv, b_qkv, w_proj, b_proj)

    in_maps = make_in_maps(x, w_qkv, b_qkv, w_proj)
    if "nc" not in _CACHE:
        _CACHE["nc"] = build_nc()
    res = run_bass_kernel_spmd(_CACHE["nc"], in_maps,
                               core_ids=list(range(NCORES)))
    outs = [np.asarray(r["y"], np.float32) for r in res.results]
    y = np.empty((B, N, D), np.float32)
    for b in range(B):
        y[b] = outs[4 * b] + outs[4 * b + 1] + outs[4 * b + 2] + outs[4 * b + 3]
    # bias: b_k cancels in softmax; b_v shifts attention output by a
    # constant -> y += b_v @ w_proj; plus the projection bias
    y += b_qkv[2 * D:] @ np.asarray(w_proj, np.float32)
    y += np.asarray(b_proj, np.float32)
    return y
